# revision 27
# baseline (speedup 1.0000x reference)
"""Multi-head attention (B=2, S=2048, D=1024, H=16) on 8 Trainium2 NeuronCores.

Sharding: data-parallel on batch, tensor-parallel on heads.
Core c handles batch b = c // 4 and heads [4*(c%4), 4*(c%4)+4).
Each core computes its 4 heads' attention + its partial Wo projection;
the host sums the 4 partial [S, D] outputs per batch (the TP all-reduce).

Device-side layout choices (see comments inline):
- Host pre-transposes query/context to [D, S] (bf16) so every matmul
  contraction dim lands on SBUF partitions with no on-device transposes.
- Scores are computed transposed ([c, q]) so the PV matmul consumes the
  exp'd probabilities directly as the moving operand, and the softmax
  denominator comes free as a 65th row of the PV output (ones column in V).
- bf16 storage/matmul operands, fp32 PSUM accumulation throughout.
"""

import numpy as np
import ml_dtypes

import concourse.bacc as bacc
import concourse.mybir as mybir
from concourse.tile import TileContext
from concourse.bass_utils import run_bass_kernel_spmd

BF16 = mybir.dt.bfloat16
FP8 = mybir.dt.float8e4
F32 = mybir.dt.float32

B, S, D, H = 2, 2048, 1024, 16
SPH = D // H          # 64
NH = 4                # heads per core
P = 128               # SBUF partitions
DC = D // P           # 8 d-chunks
CT = S // P           # 16 c-tiles
QT = S // P           # 16 q-tiles
NEG_INF = -1e9

_NC_CACHE = {}


def _build(masked: bool):
    nc = bacc.Bacc("TRN2", target_bir_lowering=False, debug=False, num_devices=8)

    qt_d = nc.declare_dram_parameter("qt", [D, S], BF16, isOutput=False)
    ct_d = nc.declare_dram_parameter("ctx", [D, S], BF16, isOutput=False)
    wq_d = nc.declare_dram_parameter("wq", [D, NH * SPH], BF16, isOutput=False)
    wk_d = nc.declare_dram_parameter("wk", [D, NH * SPH], BF16, isOutput=False)
    wv_d = nc.declare_dram_parameter("wv", [D, NH * SPH], BF16, isOutput=False)
    wo_d = nc.declare_dram_parameter("wo", [NH * SPH, D], BF16, isOutput=False)
    if masked:
        mk_d = nc.declare_dram_parameter("maskT", [S, S], BF16, isOutput=False)
    out_d = nc.declare_dram_parameter("out", [S, D], BF16, isOutput=True)

    with TileContext(nc) as tc:
        with (
            tc.tile_pool(name="const", bufs=1) as const,
            tc.tile_pool(name="work", bufs=1) as work,
            tc.tile_pool(name="pt", bufs=3) as ptp,
            tc.tile_pool(name="outp", bufs=2) as outp,
            tc.tile_pool(name="psS", bufs=2, space="PSUM") as psS,
            tc.tile_pool(name="psA", bufs=2, space="PSUM") as psA,
        ):
            # ---- stage inputs in SBUF -------------------------------------
            # weights first (small), then the 4MB inputs in 2-chunk pieces so
            # projection matmuls can start as chunks land
            wq_sb = const.tile([P, DC, NH * SPH], BF16)
            nc.sync.dma_start(out=wq_sb, in_=wq_d[:, :].rearrange("(c p) n -> p c n", p=P))
            wk_sb = const.tile([P, DC, NH * SPH], BF16)
            nc.sync.dma_start(out=wk_sb, in_=wk_d[:, :].rearrange("(c p) n -> p c n", p=P))
            wv_sb = const.tile([P, DC, NH * SPH], BF16)
            nc.sync.dma_start(out=wv_sb, in_=wv_d[:, :].rearrange("(c p) n -> p c n", p=P))
            # wo rows are (h, s); head pair t = h//2 packs two heads into the
            # partition dim (head h%2==0 -> partitions 0-63, ==1 -> 64-127).
            wo_sb = const.tile([P, 2, D], BF16)
            nc.sync.dma_start(out=wo_sb, in_=wo_d[:, :].rearrange("(t x) d -> x t d", x=P))
            qt_sb = const.tile([P, DC, S], BF16)
            qt_r = qt_d[:, :].rearrange("(c p) q -> p c q", p=P)
            ct_sb = const.tile([P, DC, S], BF16)
            ct_r = ct_d[:, :].rearrange("(c p) q -> p c q", p=P)
            for i in range(0, DC, 2):
                nc.sync.dma_start(out=ct_sb[:, i:i + 2, :], in_=ct_r[:, i:i + 2, :])
            for i in range(0, DC, 2):
                nc.sync.dma_start(out=qt_sb[:, i:i + 2, :], in_=qt_r[:, i:i + 2, :])

            # exp bias constant (see attention loop)
            expb = const.tile([P, 1], F32, name="expb")
            nc.vector.memset(expb, -1.5)

            # ---- projections: qT/kT [64, S] per head, packed per pair -----
            qTp = [work.tile([P, S], BF16, tag=f"qT{p}", name=f"qT{p}") for p in range(2)]
            kTp = [work.tile([P, S], BF16, tag=f"kT{p}", name=f"kT{p}") for p in range(2)]

            def emit_proj_chunk(p, which, qc4):
                src_sb = wk_sb if which == "k" else wq_sb
                x_sb = ct_sb if which == "k" else qt_sb
                dst = kTp[p] if which == "k" else qTp[p]
                ps = psS.tile([P, 512], F32, tag="S", bufs=2, name="ps")
                for dc in range(DC):
                    nc.tensor.matmul(
                        ps[:, :],
                        src_sb[:, dc, P * p:P * (p + 1)],
                        x_sb[:, dc, 512 * qc4:512 * (qc4 + 1)],
                        start=(dc == 0), stop=(dc == DC - 1),
                    )
                nc.scalar.copy(dst[:, 512 * qc4:512 * (qc4 + 1)], ps[:, :])

            def emit_proj(p):
                for which in ("k", "q"):
                    for qc4 in range(4):
                        emit_proj_chunk(p, which, qc4)

            # ---- V in natural [c, (h, s)] layout + ones column ------------
            vaug = work.tile([P, CT // 2, NH, 2, 80], BF16)
            nc.vector.memset(vaug[:, :, :, :, SPH:SPH + 1], 1.0)

            def emit_v():
                for ct in range(CT):
                    psv = psS.tile([P, NH * SPH], F32, tag="S", bufs=2, name="psv")
                    for dc in range(DC):
                        nc.tensor.matmul(
                            psv[:, :],
                            ct_sb[:, dc, P * ct:P * (ct + 1)],
                            wv_sb[:, dc, :],
                            start=(dc == 0), stop=(dc == DC - 1),
                        )
                    nc.vector.tensor_copy(
                        vaug[:, ct // 2, :, ct % 2, 0:SPH],
                        psv[:, :].rearrange("p (h s) -> p h s", h=NH),
                    )

            # ---- attention, two heads of a pair interleaved ---------------
            # Heads a=0 / a=1 of a pair live at partition bases 0 / 64, so
            # their score matmuls target different PE row groups and run
            # concurrently when emitted adjacently -- into different banks of
            # ONE S tile [128, 2, 512]. One exp op then covers both heads.
            # PSUM: S [2 banks]x2bufs + po [1 bank]x4 = 8 banks exactly.
            # outT_qc[qc]: [(a*64+s) partition, pair, 1024 q] bf16
            outT_qc = [work.tile([P, 2, 1024], BF16, tag=f"oT{qc}", name=f"oT{qc}") for qc in range(2)]

            def epilogue(po, p, a, qc4):
                # normalize rows 0-63 by reciprocal of row 64. Engines cannot
                # shift partitions: denominator row goes PSUM -> SBUF (DVE,
                # base-matched), row 64 -> row 0 via DMA, then gpsimd
                # broadcast (reads partition 0 only), recip, multiply.
                qc, off = qc4 // 2, 512 * (qc4 % 2)
                srow = outp.tile([P, 512], F32, tag="srow", name="srow")
                nc.vector.tensor_copy(srow[SPH:SPH + 1, :], po[SPH:SPH + 1, :])
                drow = outp.tile([1, 512], F32, tag="drow", name="drow")
                nc.sync.dma_start(out=drow[0:1, :], in_=srow[SPH:SPH + 1, :])
                rb = outp.tile([SPH, 512], F32, tag="rb", name="rb")
                nc.gpsimd.partition_broadcast(rb, drow[0:1, :], channels=SPH)
                rb2 = outp.tile([SPH, 512], F32, tag="rb2", name="rb2")
                nc.vector.reciprocal_approx_fast(rb2, rb)
                if a == 0:
                    nc.vector.tensor_mul(outT_qc[qc][0:SPH, p, off:off + 512],
                                         po[0:SPH, :], rb2)
                else:
                    ot = ptp.tile([SPH, 512], BF16, tag="ott", name="ot")
                    nc.vector.tensor_mul(ot, po[0:SPH, :], rb2)
                    # partition shift 0-63 -> 64-127 has to go through DMA
                    nc.sync.dma_start(out=outT_qc[qc][SPH:P, p, off:off + 512], in_=ot)

            def emit_wo(qc4, tail=False):
                # output projection for one 512-wide q chunk, overlapping the
                # next chunk's attention. Concurrent row-group matmuls may not
                # accumulate into the same PSUM bank (HW hang): one
                # accumulator per row group, DVE adds them.
                qc = qc4 // 2
                for qt4 in range(4):
                    qt = 4 * qc4 + qt4
                    off = (qt % 8) * P
                    osb = outp.tile([P, D], BF16, tag="osb", name="osb")
                    for dh in range(2):
                        wops0 = psA.tile([P, 512], F32, tag="A", name="wops0", bufs=4)
                        wops1 = psA.tile([P, 512], F32, tag="A", name="wops1", bufs=4)
                        for p in range(2):
                            for a in range(2):
                                lo, hi = SPH * a, SPH * (a + 1)
                                wx = wops0 if a == 0 else wops1
                                nc.tensor.matmul(
                                    wx[:, :],
                                    outT_qc[qc][lo:hi, p, off:off + P],
                                    wo_sb[lo:hi, p, 512 * dh:512 * (dh + 1)],
                                    start=(p == 0), stop=(p == 1))
                        tcp = outp.tile([P, 512], F32, tag="tcp", name="tcp")
                        if tail:
                            nc.scalar.copy(tcp, wops1)
                        else:
                            nc.vector.tensor_copy(tcp, wops1)
                        nc.vector.tensor_add(osb[:, 512 * dh:512 * (dh + 1)],
                                             wops0, tcp)
                    nc.sync.dma_start(out=out_d[P * qt:P * (qt + 1), :], in_=osb)

            def attn_block(qc4, p):
                q0 = 512 * qc4
                poAB = [psA.tile([SPH + 1, 512], F32, tag="A", name=f"po{a}",
                                 bufs=4)
                        for a in range(2)]
                pend = None
                for ct in range(CT + 1):
                    if ct < CT:
                        Sp = psS.tile([P, 2, 512], F32, tag="S", name="Sp",
                                      bufs=2)
                        for a in range(2):
                            lo, hi = SPH * a, SPH * (a + 1)
                            nc.tensor.matmul(
                                Sp[:, a, :],
                                kTp[p][lo:hi, P * ct:P * (ct + 1)],
                                qTp[p][lo:hi, q0:q0 + 512],
                                start=True, stop=True)
                        if masked:
                            mk = ptp.tile([P, 512], BF16, tag="mk", name="mk")
                            nc.sync.dma_start(
                                out=mk,
                                in_=mk_d[P * ct:P * (ct + 1), q0:q0 + 512])
                            for a in range(2):
                                nc.vector.tensor_add(Sp[:, a, :], Sp[:, a, :], mk)
                    if ct >= 1:
                        pct = ct - 1
                        PT = ptp.tile([P, 2, 512], BF16, tag="PT", name="PT")
                        nc.scalar.activation(
                            PT[:, :, :], pend[:, :, :],
                            mybir.ActivationFunctionType.Exp)
                        for a in range(2):
                            nc.tensor.matmul(
                                poAB[a][:, :],
                                vaug[:, pct // 2, 2 * p + a, pct % 2, 0:SPH + 1],
                                PT[:, a, :],
                                start=(pct == 0), stop=(pct == CT - 1))
                    if ct < CT:
                        pend = Sp
                for a in range(2):
                    epilogue(poAB[a], p, a, qc4)

            # pair-0 blocks run first (ACT-bound); pair-1 projections are
            # woven between them so the PE slack absorbs them. wo(qc4)
            # follows each pair-1 block.
            for qc4 in range(4):
                emit_proj_chunk(0, "k", qc4)
            emit_v()
            emit_proj_chunk(0, "q", 0)
            attn_block(0, 0)
            for qc4 in range(1, 4):
                emit_proj_chunk(0, "q", qc4)
            for qc4 in range(4):
                emit_proj_chunk(1, "k", qc4)
            attn_block(1, 0)
            for qc4 in range(2):
                emit_proj_chunk(1, "q", qc4)
            attn_block(2, 0)
            for qc4 in range(2, 4):
                emit_proj_chunk(1, "q", qc4)
            attn_block(3, 0)
            for qc4 in range(4):
                attn_block(qc4, 1)
                emit_wo(qc4, tail=(qc4 == 3))

    nc.compile()
    return nc


def _get_nc(masked: bool):
    if masked not in _NC_CACHE:
        _NC_CACHE[masked] = _build(masked)
    return _NC_CACHE[masked]


def kernel(query, context, attention_mask, Wq, Wk, Wv, Wo, **_unused):
    query = np.asarray(query, dtype=np.float32)
    context = np.asarray(context, dtype=np.float32)
    attention_mask = np.asarray(attention_mask, dtype=np.float32)
    Wq = np.asarray(Wq, dtype=np.float32)
    Wk = np.asarray(Wk, dtype=np.float32)
    Wv = np.asarray(Wv, dtype=np.float32)
    Wo = np.asarray(Wo, dtype=np.float32)

    masked = bool(np.any(attention_mask))
    nc = _get_nc(masked)

    bf = ml_dtypes.bfloat16
    # fold the 1/sqrt(SPH) score scale into Wq
    wq_s = (Wq * (SPH ** -0.5)).astype(bf)
    wk_s = Wk.astype(bf)
    wv_s = Wv.astype(bf)
    wo_s = Wo.astype(bf)

    qtT = [np.ascontiguousarray(query[b].T).astype(bf) for b in range(B)]
    ctT = [np.ascontiguousarray(context[b].T).astype(bf) for b in range(B)]
    if masked:
        mkT = [np.ascontiguousarray((attention_mask[b, 0] * NEG_INF).T).astype(bf)
               for b in range(B)]

    in_maps = []
    for c in range(8):
        b, g = c // 4, c % 4
        hs = slice(NH * g, NH * (g + 1))
        im = {
            "qt": qtT[b],
            "ctx": ctT[b],
            "wq": np.ascontiguousarray(wq_s[:, hs, :]).reshape(D, NH * SPH),
            "wk": np.ascontiguousarray(wk_s[:, hs, :]).reshape(D, NH * SPH),
            "wv": np.ascontiguousarray(wv_s[:, hs, :]).reshape(D, NH * SPH),
            "wo": np.ascontiguousarray(wo_s[hs]).reshape(NH * SPH, D),
        }
        if masked:
            im["maskT"] = mkT[b]
        in_maps.append(im)

    res = run_bass_kernel_spmd(nc, in_maps, core_ids=list(range(8)))

    out = np.zeros((B, S, D), dtype=np.float32)
    for c in range(8):
        out[c // 4] += res.results[c]["out"].astype(np.float32)
    return out
